# revision 12
# baseline (speedup 1.0000x reference)
"""Trainium2 Bass kernel for nn_Attention_10917806866815.

Multi-head attention forward (B=8, S=32x32=1024, C=768, 12 heads, hd=64),
data-parallel across 8 NeuronCores: core b computes batch element b.
No collectives needed.

Host side: inputs pre-transposed to channel-major fp16, so the device
kernel is pure matmul work: xT [768,1024], w_qkvT [768,2304],
w_projT [768,768].

Per-core pipeline, software-pipelined around the two binding engines
(TensorE matmul stream ~125us of N-cycles, ScalarE exp stream ~106us):
  - 8 consolidated 3D DMAs; a long zero-matmul warmup keeps the PE
    clock ramped through the DMA lead-in.
  - Per head pair p, per token-tile T (slot): 4 score matmuls (the two
    heads live at partition bases 0/64 so their K=64 matmuls run on
    disjoint PE row groups concurrently), 2 exps (ScalarE, straight out
    of PSUM; scores ~ N(0,1)*8 pre-scale so exp cannot overflow), then
    background PE work: PV chunk-0 at one-slot lag, the previous pair's
    PV chunk-1 spread over slots 0-3 (pt tiles are double-buffered so
    they survive into the next pair), and the next pair's q/k
    projections + v token-tiles as slot extras (slots 4-7).
  - PV uses 65-wide lhsT = [v_h | ones]: attn rows 0-63 plus softmax
    sums in row 64. Sums evacuate to a [2, S] tile on GpSimd (idle
    engine); attn rows on DVE.
  - Normalization per half: DVE reciprocal of [2,512] sums, bounce via
    DRAM for the partition-broadcast, one in-place multiply. No ScalarE
    table swap (keeps the exp table resident).
  - Output projection from the c-major attnT tiles, fp16 out DMA;
    bias is added host-side.

Precision: fp16 operands (10-bit mantissa) with fp32 PSUM accumulation.
"""

import numpy as np

import concourse.bass as bass
import concourse.mybir as mybir
import concourse.tile as tile
from concourse import bacc
from concourse.bass_utils import run_bass_kernel_spmd

DIM = 768
S = 1024
NH = 12
HD = 64
SCALE = HD ** -0.5

F32 = mybir.dt.float32
FP16 = mybir.dt.float16

NC_T = S // 128          # 8 token tiles
NC_C = DIM // 128        # 6 channel tiles
NPAIR = NH // 2          # 6 head pairs
VW = HD + 1              # 65: v columns per head incl. ones column

N_WARMUP = 6


def build_bass():
    nc = bacc.Bacc(None, target_bir_lowering=False)

    xT_ext = nc.declare_dram_parameter("xT", [DIM, S], FP16, isOutput=False)
    wqkvT_ext = nc.declare_dram_parameter(
        "w_qkvT", [DIM, 3 * DIM], FP16, isOutput=False
    )
    wprojT_ext = nc.declare_dram_parameter(
        "w_projT", [DIM, DIM], FP16, isOutput=False
    )
    out_ext = nc.declare_dram_parameter("out", [S, DIM], FP16, isOutput=True)

    with tile.TileContext(nc) as tc:
        from contextlib import ExitStack

        with ExitStack() as ctx:
            consts = ctx.enter_context(tc.tile_pool(name="consts", bufs=1))
            persist = ctx.enter_context(tc.tile_pool(name="persist", bufs=1))

            # c-major operands: [:, j, :] is channel-tile j.
            xT = persist.tile([128, NC_C, S], FP16, tag="xT", name="xT")
            wqkvT = persist.tile(
                [128, NC_C, 3 * DIM], FP16, tag="wqkvT", name="wqkvT"
            )
            wprojT = persist.tile([128, NC_C, DIM], FP16, tag="wprojT", name="wprojT")

            # warmup operands + fp32 ones row (PE-broadcast normalization of
            # the last pair); emitted before the DMAs so the warmup matmuls
            # start the moment the framework preamble ends
            wu = consts.tile([128, 512], FP16, tag="wu", name="wu")
            nc.vector.memset(wu[:], 0.0)
            ones_f32 = consts.tile([1, 64], F32, tag="ones_f32", name="ones_f32")
            nc.vector.memset(ones_f32[:], 1.0)

            # one 3D DMA per column range: dst [128, j, cols] <- src rows
            # (j*128 + p), ordered so pair-0's q/k data lands first
            def dma_cols(dst, src, src_w, lo, hi):
                src3 = src.rearrange("(j p) s -> p j s", p=128)
                nc.sync.dma_start(
                    out=dst[:, :, lo:hi],
                    in_=src3[:, :, lo:hi],
                )

            # lead-in data split per channel-tile so the first q/k
            # projection chain pipelines with the DMA stream
            dma_cols(wqkvT, wqkvT_ext, 3 * DIM, 0, 128)
            dma_cols(wqkvT, wqkvT_ext, 3 * DIM, DIM, DIM + 128)
            for j in range(NC_C):
                nc.sync.dma_start(
                    out=xT[:, j, 0:512],
                    in_=xT_ext[j * 128:(j + 1) * 128, 0:512],
                )
            dma_cols(xT, xT_ext, S, 512, 1024)
            dma_cols(wqkvT, wqkvT_ext, 3 * DIM, 2 * DIM, 3 * DIM)
            dma_cols(wqkvT, wqkvT_ext, 3 * DIM, 128, DIM)
            dma_cols(wqkvT, wqkvT_ext, 3 * DIM, DIM + 128, 2 * DIM)
            dma_cols(wprojT, wprojT_ext, DIM, 0, DIM)

            qkT = [
                persist.tile([128, S], FP16, tag=f"qkT{ot}", name=f"qkT{ot}")
                for ot in range(2 * NPAIR)
            ]
            v_ext = [
                persist.tile([128, NH * VW], FP16, tag=f"vext{tt}", name=f"vext{tt}")
                for tt in range(NC_T)
            ]
            attnT = [
                persist.tile([128, S], FP16, tag=f"attnT{p}", name=f"attnT{p}")
                for p in range(NPAIR)
            ]
            for tt in range(NC_T):
                nc.gpsimd.memset(v_ext[tt][:], 1.0)

            with (
                tc.tile_pool(name="stps", bufs=1, space="PSUM") as stps,
                tc.tile_pool(name="pvps", bufs=1, space="PSUM") as pvps,
                tc.tile_pool(name="bgps", bufs=1, space="PSUM") as bgps,
                tc.tile_pool(name="ptpool", bufs=1) as ptpool,
                tc.tile_pool(name="normp", bufs=2) as normp,
                tc.tile_pool(name="outp", bufs=3) as outp,
                tc.tile_pool(name="rdram", bufs=2, space="DRAM") as rdram,
            ):
                # background PSUM: two [128,512] banks time-shared by the
                # QKV-projection extras (slots 4-7) and the spread PV
                # chunk-1 accumulators (slots 0-3)
                bg_flip = [0]

                def bg_tile(name):
                    t = bgps.tile([128, 512], F32, tag=f"bg{bg_flip[0]}",
                                  name=name, bufs=1)
                    bg_flip[0] ^= 1
                    return t

                # ---- QKV building blocks (emitted as in-slot extras) ----
                def emit_qk_chunk(ot, c):
                    ps = bg_tile("qkvp")
                    for k in range(NC_C):
                        nc.tensor.matmul(
                            ps[:],
                            wqkvT[:, k, ot * 128:(ot + 1) * 128],
                            xT[:, k, c * 512:(c + 1) * 512],
                            start=(k == 0),
                            stop=(k == NC_C - 1),
                        )
                    nc.vector.tensor_copy(qkT[ot][:, c * 512:(c + 1) * 512], ps[:])

                def emit_v_chunk(tt, c):
                    o0, ow, h0, nh = [
                        (2 * DIM, 512, 0, 8), (2 * DIM + 512, 256, 8, 4)
                    ][c]
                    ps = bg_tile("vp")
                    for k in range(NC_C):
                        nc.tensor.matmul(
                            ps[:, :ow],
                            xT[:, k, tt * 128:(tt + 1) * 128],
                            wqkvT[:, k, o0:o0 + ow],
                            start=(k == 0),
                            stop=(k == NC_C - 1),
                        )
                    dst = (
                        v_ext[tt][:]
                        .rearrange("p (h e) -> p h e", e=VW)[:, h0:h0 + nh, 0:HD]
                    )
                    nc.vector.tensor_copy(
                        dst, ps[:, :ow].rearrange("p (h e) -> p h e", e=HD)
                    )

                # extras[p][T]: QKV work slotted into PE idle time under the
                # exp stream. Pair 0 carries its own v c0 tiles (slots 0-7)
                # plus pair 1's q/k; later pairs carry q/k for pair p+1 and
                # the v c1 tiles (slots 4-7 only -- the bg PSUM banks are
                # held by the spread PV chunk-1 during slots 0-3).
                extras = [[[] for _ in range(NC_T)] for _ in range(NPAIR)]

                def TH(f, *a):
                    return lambda: f(*a)

                for tt in range(NC_T):
                    extras[0][tt].append(TH(emit_v_chunk, tt, 0))
                extras[0][2].append(TH(emit_qk_chunk, NPAIR, 1))
                for p in range(NPAIR - 1):  # q/k for pair p+1
                    extras[p][4].append(TH(emit_qk_chunk, p + 1, 0))
                    extras[p][4].append(TH(emit_qk_chunk, NPAIR + p + 1, 0))
                    extras[p][5].append(TH(emit_qk_chunk, p + 1, 1))
                    extras[p][5].append(TH(emit_qk_chunk, NPAIR + p + 1, 1))
                # v chunk-1 tiles: pairs 1-4, slot 6 (v[k] c1 is first
                # read by pv0 of pair 4 at slot k+1; slot 7 stays free so
                # the pair transition flows straight into the next scores)
                for i in range(NC_T):
                    extras[1 + i // 2][6].append(TH(emit_v_chunk, i, 1))

                # ---- HAM warm-up: keep the PE busy through the DMA
                # lead-in so the first real matmuls run at full clock ----
                wups = bg_tile("wups")
                for i in range(N_WARMUP):
                    nc.tensor.matmul(
                        wups[:], wu[:, 0:128], wu[:],
                        start=(i == 0), stop=(i == N_WARMUP - 1),
                    )

                # ---- pair-0 q/k projection, c0 only: slot (0,0)'s
                # T=0 scores touch kT cols 0-127 and both q halves; q c1 is
                # projected just-in-time inside the split slot, k c1 (first
                # needed at T=4) rides as a slot-2 extra ----
                emit_qk_chunk(0, 0)
                emit_qk_chunk(NPAIR, 0)

                # ---- attention: software-pipelined slot stream ----
                pts_of = {}     # p -> [pt]*8, pt cols = h*S + q
                pv0_of = {}     # p -> [pv0_h0, pv0_h1]
                pv1_of = {}     # p -> [pv1_h0, pv1_h1]
                sums_sb_of = {}  # p -> [2, S] sums tile (h, q)

                def emit_pv1_mm(pp, tprims):
                    # spread PV chunk-1 matmuls for pair pp
                    for h in range(2):
                        for T in tprims:
                            nc.tensor.matmul(
                                pv1_of[pp][h][0:VW, :],
                                v_ext[T][:, (2 * pp + h) * VW:(2 * pp + h + 1) * VW],
                                pts_of[pp][T][:, h * S + 512:h * S + 1024],
                                start=(T == 0),
                                stop=(T == NC_T - 1),
                            )

                def evac_pv1(pp):
                    for h in range(2):
                        nc.vector.tensor_copy(
                            sums_sb_of[pp][0:1, h * S + 512:h * S + 1024],
                            pv1_of[pp][h][HD:HD + 1, :],
                        )
                        nc.vector.tensor_copy(
                            attnT[pp][h * 64:(h + 1) * 64, 512:1024],
                            pv1_of[pp][h][0:HD, :],
                        )
                    del pv1_of[pp], pts_of[pp]
                    norm_half(pp, 1)

                def slot(p, T):
                    kT_t = qkT[NPAIR + p]
                    qT_t = qkT[p]
                    if T == 0:
                        sums_sb_of[p] = normp.tile(
                            [1, 2 * S], F32, tag="sums", name="sums", bufs=2
                        )
                        pv0_of[p] = [
                            pvps.tile([VW, 512], F32, tag=f"pva{h}",
                                      name=f"pva{h}", bufs=1)
                            for h in range(2)
                        ]
                        pts_of[p] = [None] * NC_T
                    # two per-h score tiles: h1's exp overlaps h0's
                    # next-slot scores (separate PSUM bank pairs)
                    st = [
                        stps.tile([128, S], F32, tag=f"st{h}", name=f"st{h}",
                                  bufs=1)
                        for h in range(2)
                    ]
                    pt = ptpool.tile(
                        [128, 2 * S], FP16, tag=f"pt{T}", name=f"pt{T}",
                        bufs=2
                    )
                    pts_of[p][T] = pt

                    def score_mm(c, h):
                        r0 = h * 64
                        # partition bases 0/64 -> disjoint PE row groups
                        # -> the two heads' score matmuls overlap
                        nc.tensor.matmul(
                            st[h][:, c * 512:(c + 1) * 512],
                            kT_t[r0:r0 + 64, T * 128:(T + 1) * 128],
                            qT_t[r0:r0 + 64, c * 512:(c + 1) * 512],
                            start=True,
                            stop=True,
                        )

                    def exp_emit(h, cols):
                        nc.scalar.activation(
                            out=pt[:, h * S + cols.start:h * S + cols.stop],
                            in_=st[h][:, cols],
                            func=mybir.ActivationFunctionType.Exp,
                            scale=float(SCALE),
                        )

                    if p == 0 and T == 0:
                        # pipeline-fill slot: per-half exps so ScalarE
                        # starts before the q c1 projection exists
                        for h in range(2):
                            score_mm(0, h)
                        for h in range(2):
                            exp_emit(h, slice(0, 512))
                        emit_qk_chunk(0, 1)
                        for h in range(2):
                            score_mm(1, h)
                        for h in range(2):
                            exp_emit(h, slice(512, 1024))
                    else:
                        for c in range(2):
                            for h in range(2):
                                score_mm(c, h)
                        for h in range(2):
                            exp_emit(h, slice(0, S))
                    # previous pair's PV chunk-1, spread over slots 0-3
                    if p > 0 and T <= 3:
                        if T == 0:
                            pv1_of[p - 1] = [bg_tile(f"pvb{h}") for h in range(2)]
                        emit_pv1_mm(p - 1, [2 * T, 2 * T + 1])
                        if T == 3:
                            evac_pv1(p - 1)
                    # last pair: own PV chunk-1 in slots 4-7 (bg banks are
                    # free -- no next pair's q/k to project)
                    if p == NPAIR - 1 and T >= 4:
                        if T == 4:
                            pv1_of[p] = [bg_tile(f"pvb{h}") for h in range(2)]
                        emit_pv1_mm(p, [2 * (T - 4), 2 * (T - 4) + 1])
                    for th in extras[p][T]:
                        th()
                    if T > 0:
                        for h in range(2):
                            nc.tensor.matmul(
                                pv0_of[p][h][:],
                                v_ext[T - 1][
                                    :, (2 * p + h) * VW:(2 * p + h + 1) * VW
                                ],
                                pts_of[p][T - 1][:, h * S:h * S + 512],
                                start=(T == 1),
                                stop=(T == NC_T - 1),
                            )

                def finish_c0(p):
                    # last chunk-0 PV matmul + chunk-0 evacuation
                    for h in range(2):
                        nc.tensor.matmul(
                            pv0_of[p][h][:],
                            v_ext[NC_T - 1][
                                :, (2 * p + h) * VW:(2 * p + h + 1) * VW
                            ],
                            pts_of[p][NC_T - 1][:, h * S:h * S + 512],
                            start=False,
                            stop=True,
                        )
                    for h in range(2):
                        nc.vector.tensor_copy(
                            sums_sb_of[p][0:1, h * S:h * S + 512],
                            pv0_of[p][h][HD:HD + 1, :],
                        )
                        nc.vector.tensor_copy(
                            attnT[p][h * 64:(h + 1) * 64, 0:512],
                            pv0_of[p][h][0:HD, :],
                        )
                    del pv0_of[p]
                    norm_half(p, 0)

                def norm_half(p, c):
                    # normalize attnT[p] columns [c*512, (c+1)*512):
                    # DVE fast reciprocal of the sums, then a partition-
                    # broadcast + one in-place multiply. Mid-stream pairs
                    # bounce the reciprocals through DRAM (Sync engine,
                    # hides under the PE); the last pair is on the tail
                    # critical path, so it broadcasts via an ones-matmul on
                    # the then-idle PE instead (saves the DMA round trips).
                    lo, hi = c * 512, (c + 1) * 512
                    rr = normp.tile([1, 1024], F32, tag="rr", name="rr", bufs=2)
                    for h in range(2):
                        # sums are O(100) positive fp32 -- no edge cases;
                        # ~51 ULP is far inside the fp16 noise floor
                        nc.vector.reciprocal_approx_fast(
                            out=rr[0:1, h * 512:(h + 1) * 512],
                            in_=sums_sb_of[p][0:1, h * S + lo:h * S + hi],
                        )
                    if p == NPAIR - 1:
                        bc = bg_tile("bc")
                        for h in range(2):
                            nc.tensor.matmul(
                                bc[h * 64:(h + 1) * 64, :],
                                ones_f32[0:1, :],
                                rr[0:1, h * 512:(h + 1) * 512],
                                start=True,
                                stop=True,
                            )
                        nc.vector.tensor_mul(
                            attnT[p][:, lo:hi], attnT[p][:, lo:hi], bc[:]
                        )
                        if c == 1:
                            del sums_sb_of[p]
                        return
                    rdd = rdram.tile([1, 1024], F32, tag="rdd", name="rdd")
                    nc.sync.dma_start(out=rdd[:], in_=rr[:])
                    rb = normp.tile([128, 512], F32, tag="rb", name="rb", bufs=2)
                    for h in range(2):
                        row = rdd[0:1, h * 512:(h + 1) * 512]
                        row_bc = bass.AP(
                            tensor=row.tensor,
                            offset=row.offset,
                            ap=[[0, 64]] + list(row.ap[1:]),
                        )
                        nc.sync.dma_start(
                            out=rb[h * 64:(h + 1) * 64, :], in_=row_bc
                        )
                    nc.vector.tensor_mul(
                        attnT[p][:, lo:hi], attnT[p][:, lo:hi], rb[:]
                    )
                    if c == 1:
                        del sums_sb_of[p]

                for p in range(NPAIR):
                    for T in range(NC_T):
                        slot(p, T)
                    finish_c0(p)
                # last pair: PV c1 already fully accumulated in slots 4-7
                evac_pv1(NPAIR - 1)

                # ---------------- output projection ----------------
                # PSUM comes from the score-tile tags (st0/st1), so the
                # first projection matmuls start the moment the last exp
                # frees them. Depth-2 pipeline: each tile's pair-5 matmul
                # (gated by the last normalization) is deferred past the
                # next tile's early matmuls so the PE never blocks on it.
                def proj_head(tt, h):
                    ps = stps.tile([128, DIM], F32, tag=f"st{h}",
                                   name=f"prj{tt}", bufs=1)
                    for o0, ow in [(0, 512), (512, 256)]:
                        for p in range(NPAIR - 1):
                            nc.tensor.matmul(
                                ps[:, o0:o0 + ow],
                                attnT[p][:, tt * 128:(tt + 1) * 128],
                                wprojT[:, p, o0:o0 + ow],
                                start=(p == 0),
                                stop=False,
                            )
                    return ps

                def proj_tail(tt, ps):
                    for o0, ow in [(0, 512), (512, 256)]:
                        nc.tensor.matmul(
                            ps[:, o0:o0 + ow],
                            attnT[NPAIR - 1][:, tt * 128:(tt + 1) * 128],
                            wprojT[:, NPAIR - 1, o0:o0 + ow],
                            start=False,
                            stop=True,
                        )
                    ob = outp.tile([128, DIM], FP16, tag="ob", name="ob")
                    nc.scalar.copy(out=ob[:], in_=ps[:])
                    nc.sync.dma_start(
                        out=out_ext[tt * 128:(tt + 1) * 128, :], in_=ob[:]
                    )

                pending = None
                for tt in range(NC_T):
                    ps = proj_head(tt, tt % 2)
                    if pending is not None:
                        proj_tail(*pending)
                    pending = (tt, ps)
                proj_tail(*pending)

    nc.finalize()
    return nc


_NC_CACHE = None


def kernel(**inputs) -> np.ndarray:
    global _NC_CACHE
    x = np.asarray(inputs["x"], dtype=np.float32)
    w_qkv = np.asarray(inputs["w_qkv"], dtype=np.float32)
    w_proj = np.asarray(inputs["w_proj"], dtype=np.float32)
    b_proj = np.asarray(inputs["b_proj"], dtype=np.float32)
    B, H, W, C = x.shape
    assert (B, H * W, C) == (8, S, DIM)

    # host-side sharding + layout prep: channel-major fp16 operands
    wqkvT = np.ascontiguousarray(w_qkv.T).astype(np.float16)       # [768, 2304]
    wprojT = np.ascontiguousarray(w_proj.T).astype(np.float16)     # [768, 768]
    xTs = [
        np.ascontiguousarray(x[b].reshape(S, DIM).T).astype(np.float16)
        for b in range(B)
    ]

    if _NC_CACHE is None:
        _NC_CACHE = build_bass()
    nc = _NC_CACHE

    in_maps = [
        {"xT": xTs[b], "w_qkvT": wqkvT, "w_projT": wprojT}
        for b in range(B)
    ]
    res = run_bass_kernel_spmd(nc, in_maps, list(range(B)))
    out = np.stack(
        [
            np.asarray(res.results[b]["out"]).astype(np.float32).reshape(H, W, C)
            for b in range(B)
        ]
    )
    return (out + b_proj.reshape(1, 1, 1, C)).astype(np.float32)


if __name__ == "__main__":
    rng = np.random.default_rng(0)
    ins = {
        "x": rng.standard_normal((8, 32, 32, DIM), dtype=np.float32),
        "w_qkv": rng.standard_normal((3 * DIM, DIM), dtype=np.float32)
        * DIM ** -0.5,
        "w_proj": rng.standard_normal((DIM, DIM), dtype=np.float32) * DIM ** -0.5,
        "b_proj": np.zeros(DIM, dtype=np.float32),
    }
    o = kernel(**ins)
    print(o.shape, o.dtype)


# revision 14
# speedup vs baseline: 1.0070x; 1.0070x over previous
"""Trainium2 Bass kernel for nn_Attention_10917806866815.

Multi-head attention forward (B=8, S=32x32=1024, C=768, 12 heads, hd=64),
data-parallel across 8 NeuronCores: core b computes batch element b.
No collectives needed.

Host side: inputs pre-transposed to channel-major fp16, so the device
kernel is pure matmul work: xT [768,1024], w_qkvT [768,2304],
w_projT [768,768].

Per-core pipeline, software-pipelined around the two binding engines
(TensorE matmul stream ~125us of N-cycles, ScalarE exp stream ~106us):
  - 8 consolidated 3D DMAs; a long zero-matmul warmup keeps the PE
    clock ramped through the DMA lead-in.
  - Per head pair p, per token-tile T (slot): 4 score matmuls (the two
    heads live at partition bases 0/64 so their K=64 matmuls run on
    disjoint PE row groups concurrently), 2 exps (ScalarE, straight out
    of PSUM; scores ~ N(0,1)*8 pre-scale so exp cannot overflow), then
    background PE work: PV chunk-0 at one-slot lag, the previous pair's
    PV chunk-1 spread over slots 0-3 (pt tiles are double-buffered so
    they survive into the next pair), and the next pair's q/k
    projections + v token-tiles as slot extras (slots 4-7).
  - PV uses 65-wide lhsT = [v_h | ones]: attn rows 0-63 plus softmax
    sums in row 64. Sums evacuate to a [2, S] tile on GpSimd (idle
    engine); attn rows on DVE.
  - Normalization per half: DVE reciprocal of [2,512] sums, bounce via
    DRAM for the partition-broadcast, one in-place multiply. No ScalarE
    table swap (keeps the exp table resident).
  - Output projection from the c-major attnT tiles, fp16 out DMA;
    bias is added host-side.

Precision: fp16 operands (10-bit mantissa) with fp32 PSUM accumulation.
"""

import numpy as np

import concourse.bass as bass
import concourse.mybir as mybir
import concourse.tile as tile
from concourse import bacc
from concourse.bass_utils import run_bass_kernel_spmd

DIM = 768
S = 1024
NH = 12
HD = 64
SCALE = HD ** -0.5

F32 = mybir.dt.float32
FP16 = mybir.dt.float16

NC_T = S // 128          # 8 token tiles
NC_C = DIM // 128        # 6 channel tiles
NPAIR = NH // 2          # 6 head pairs
VW = HD + 1              # 65: v columns per head incl. ones column

N_WARMUP = 6


def build_bass():
    nc = bacc.Bacc(None, target_bir_lowering=False)

    xT_ext = nc.declare_dram_parameter("xT", [DIM, S], FP16, isOutput=False)
    wqkvT_ext = nc.declare_dram_parameter(
        "w_qkvT", [DIM, 3 * DIM], FP16, isOutput=False
    )
    wprojT_ext = nc.declare_dram_parameter(
        "w_projT", [DIM, DIM], FP16, isOutput=False
    )
    out_ext = nc.declare_dram_parameter("out", [S, DIM], FP16, isOutput=True)

    with tile.TileContext(nc) as tc:
        from contextlib import ExitStack

        with ExitStack() as ctx:
            consts = ctx.enter_context(tc.tile_pool(name="consts", bufs=1))
            persist = ctx.enter_context(tc.tile_pool(name="persist", bufs=1))

            # c-major operands: [:, j, :] is channel-tile j.
            xT = persist.tile([128, NC_C, S], FP16, tag="xT", name="xT")
            wqkvT = persist.tile(
                [128, NC_C, 3 * DIM], FP16, tag="wqkvT", name="wqkvT"
            )
            wprojT = persist.tile([128, NC_C, DIM], FP16, tag="wprojT", name="wprojT")

            # warmup operands + fp32 ones row (PE-broadcast normalization of
            # the last pair); emitted before the DMAs so the warmup matmuls
            # start the moment the framework preamble ends
            wu = consts.tile([128, 512], FP16, tag="wu", name="wu")
            nc.vector.memset(wu[:], 0.0)
            ones_f32 = consts.tile([1, 64], F32, tag="ones_f32", name="ones_f32")
            nc.vector.memset(ones_f32[:], 1.0)
            # preload the exp activation table while DMAs stream in, so the
            # first real exp doesn't pay the ~1.3us ACT_TABLE_LOAD
            tl = consts.tile([1, 8], FP16, tag="tl", name="tl")
            nc.scalar.activation(
                out=tl[:], in_=wu[0:1, 0:8],
                func=mybir.ActivationFunctionType.Exp, scale=1.0,
            )

            # one 3D DMA per column range: dst [128, j, cols] <- src rows
            # (j*128 + p), ordered so pair-0's q/k data lands first
            def dma_cols(dst, src, src_w, lo, hi):
                src3 = src.rearrange("(j p) s -> p j s", p=128)
                nc.sync.dma_start(
                    out=dst[:, :, lo:hi],
                    in_=src3[:, :, lo:hi],
                )

            # lead-in data split per channel-tile so the first q/k
            # projection chain pipelines with the DMA stream
            dma_cols(wqkvT, wqkvT_ext, 3 * DIM, 0, 128)
            dma_cols(wqkvT, wqkvT_ext, 3 * DIM, DIM, DIM + 128)
            for j in range(NC_C):
                nc.sync.dma_start(
                    out=xT[:, j, 0:512],
                    in_=xT_ext[j * 128:(j + 1) * 128, 0:512],
                )
            dma_cols(xT, xT_ext, S, 512, 1024)
            dma_cols(wqkvT, wqkvT_ext, 3 * DIM, 2 * DIM, 3 * DIM)
            dma_cols(wqkvT, wqkvT_ext, 3 * DIM, 128, DIM)
            dma_cols(wqkvT, wqkvT_ext, 3 * DIM, DIM + 128, 2 * DIM)
            dma_cols(wprojT, wprojT_ext, DIM, 0, DIM)

            qkT = [
                persist.tile([128, S], FP16, tag=f"qkT{ot}", name=f"qkT{ot}")
                for ot in range(2 * NPAIR)
            ]
            v_ext = [
                persist.tile([128, NH * VW], FP16, tag=f"vext{tt}", name=f"vext{tt}")
                for tt in range(NC_T)
            ]
            attnT = [
                persist.tile([128, S], FP16, tag=f"attnT{p}", name=f"attnT{p}")
                for p in range(NPAIR)
            ]
            for tt in range(NC_T):
                nc.gpsimd.memset(v_ext[tt][:], 1.0)

            with (
                tc.tile_pool(name="stps", bufs=1, space="PSUM") as stps,
                tc.tile_pool(name="pvps", bufs=1, space="PSUM") as pvps,
                tc.tile_pool(name="bgps", bufs=1, space="PSUM") as bgps,
                tc.tile_pool(name="ptpool", bufs=1) as ptpool,
                tc.tile_pool(name="normp", bufs=2) as normp,
                tc.tile_pool(name="outp", bufs=3) as outp,
                tc.tile_pool(name="rdram", bufs=2, space="DRAM") as rdram,
            ):
                # background PSUM: two [128,512] banks time-shared by the
                # QKV-projection extras (slots 4-7) and the spread PV
                # chunk-1 accumulators (slots 0-3)
                bg_flip = [0]

                def bg_tile(name):
                    t = bgps.tile([128, 512], F32, tag=f"bg{bg_flip[0]}",
                                  name=name, bufs=1)
                    bg_flip[0] ^= 1
                    return t

                # ---- QKV building blocks (emitted as in-slot extras) ----
                def emit_qk_chunk(ot, c):
                    ps = bg_tile("qkvp")
                    for k in range(NC_C):
                        nc.tensor.matmul(
                            ps[:],
                            wqkvT[:, k, ot * 128:(ot + 1) * 128],
                            xT[:, k, c * 512:(c + 1) * 512],
                            start=(k == 0),
                            stop=(k == NC_C - 1),
                        )
                    nc.vector.tensor_copy(qkT[ot][:, c * 512:(c + 1) * 512], ps[:])

                def emit_v_chunk(tt, c):
                    o0, ow, h0, nh = [
                        (2 * DIM, 512, 0, 8), (2 * DIM + 512, 256, 8, 4)
                    ][c]
                    ps = bg_tile("vp")
                    for k in range(NC_C):
                        nc.tensor.matmul(
                            ps[:, :ow],
                            xT[:, k, tt * 128:(tt + 1) * 128],
                            wqkvT[:, k, o0:o0 + ow],
                            start=(k == 0),
                            stop=(k == NC_C - 1),
                        )
                    dst = (
                        v_ext[tt][:]
                        .rearrange("p (h e) -> p h e", e=VW)[:, h0:h0 + nh, 0:HD]
                    )
                    nc.vector.tensor_copy(
                        dst, ps[:, :ow].rearrange("p (h e) -> p h e", e=HD)
                    )

                # extras[p][T]: QKV work slotted into PE idle time under the
                # exp stream. Pair 0 carries its own v c0 tiles (slots 0-7)
                # plus pair 1's q/k; later pairs carry q/k for pair p+1 and
                # the v c1 tiles (slots 4-7 only -- the bg PSUM banks are
                # held by the spread PV chunk-1 during slots 0-3).
                extras = [[[] for _ in range(NC_T)] for _ in range(NPAIR)]

                def TH(f, *a):
                    return lambda: f(*a)

                for tt in range(NC_T):
                    extras[0][tt].append(TH(emit_v_chunk, tt, 0))
                extras[0][2].append(TH(emit_qk_chunk, NPAIR, 1))
                for p in range(NPAIR - 1):  # q/k for pair p+1
                    extras[p][4].append(TH(emit_qk_chunk, p + 1, 0))
                    extras[p][5].append(TH(emit_qk_chunk, p + 1, 1))
                    extras[p][6].append(TH(emit_qk_chunk, NPAIR + p + 1, 0))
                    extras[p][7].append(TH(emit_qk_chunk, NPAIR + p + 1, 1))
                # v chunk-1 tiles: pairs 1-3, slots 4-6 (needed by pair 4)
                for i in range(NC_T):
                    extras[1 + i // 3][4 + i % 3].append(TH(emit_v_chunk, i, 1))

                # ---- HAM warm-up: keep the PE busy through the DMA
                # lead-in so the first real matmuls run at full clock ----
                wups = bg_tile("wups")
                for i in range(N_WARMUP):
                    nc.tensor.matmul(
                        wups[:], wu[:, 0:128], wu[:],
                        start=(i == 0), stop=(i == N_WARMUP - 1),
                    )

                # ---- pair-0 q/k projection, c0 only: slot (0,0)'s
                # T=0 scores touch kT cols 0-127 and both q halves; q c1 is
                # projected just-in-time inside the split slot, k c1 (first
                # needed at T=4) rides as a slot-2 extra ----
                emit_qk_chunk(0, 0)
                emit_qk_chunk(NPAIR, 0)

                # ---- attention: software-pipelined slot stream ----
                pts_of = {}     # p -> [pt]*8, pt cols = h*S + q
                pv0_of = {}     # p -> [pv0_h0, pv0_h1]
                pv1_of = {}     # p -> [pv1_h0, pv1_h1]
                sums_sb_of = {}  # p -> [2, S] sums tile (h, q)

                def emit_pv1_mm(pp, tprims):
                    # spread PV chunk-1 matmuls for pair pp
                    for h in range(2):
                        for T in tprims:
                            nc.tensor.matmul(
                                pv1_of[pp][h][0:VW, :],
                                v_ext[T][:, (2 * pp + h) * VW:(2 * pp + h + 1) * VW],
                                pts_of[pp][T][:, h * S + 512:h * S + 1024],
                                start=(T == 0),
                                stop=(T == NC_T - 1),
                            )

                def evac_pv1(pp):
                    for h in range(2):
                        nc.vector.tensor_copy(
                            sums_sb_of[pp][0:1, h * S + 512:h * S + 1024],
                            pv1_of[pp][h][HD:HD + 1, :],
                        )
                        nc.vector.tensor_copy(
                            attnT[pp][h * 64:(h + 1) * 64, 512:1024],
                            pv1_of[pp][h][0:HD, :],
                        )
                    del pv1_of[pp], pts_of[pp]
                    norm_half(pp, 1)

                def slot(p, T):
                    kT_t = qkT[NPAIR + p]
                    qT_t = qkT[p]
                    if T == 0:
                        sums_sb_of[p] = normp.tile(
                            [1, 2 * S], F32, tag="sums", name="sums", bufs=2
                        )
                        pv0_of[p] = [
                            pvps.tile([VW, 512], F32, tag=f"pva{h}",
                                      name=f"pva{h}", bufs=1)
                            for h in range(2)
                        ]
                        pts_of[p] = [None] * NC_T
                    # two per-h score tiles: h1's exp overlaps h0's
                    # next-slot scores (separate PSUM bank pairs)
                    st = [
                        stps.tile([128, S], F32, tag=f"st{h}", name=f"st{h}",
                                  bufs=1)
                        for h in range(2)
                    ]
                    pt = ptpool.tile(
                        [128, 2 * S], FP16, tag=f"pt{T}", name=f"pt{T}",
                        bufs=2
                    )
                    pts_of[p][T] = pt

                    def score_mm(c, h):
                        r0 = h * 64
                        # partition bases 0/64 -> disjoint PE row groups
                        # -> the two heads' score matmuls overlap
                        nc.tensor.matmul(
                            st[h][:, c * 512:(c + 1) * 512],
                            kT_t[r0:r0 + 64, T * 128:(T + 1) * 128],
                            qT_t[r0:r0 + 64, c * 512:(c + 1) * 512],
                            start=True,
                            stop=True,
                        )

                    def exp_emit(h, cols):
                        nc.scalar.activation(
                            out=pt[:, h * S + cols.start:h * S + cols.stop],
                            in_=st[h][:, cols],
                            func=mybir.ActivationFunctionType.Exp,
                            scale=float(SCALE),
                        )

                    if p == 0 and T == 0:
                        # pipeline-fill slot: per-half exps so ScalarE
                        # starts before the q c1 projection exists
                        for h in range(2):
                            score_mm(0, h)
                        for h in range(2):
                            exp_emit(h, slice(0, 512))
                        emit_qk_chunk(0, 1)
                        for h in range(2):
                            score_mm(1, h)
                        for h in range(2):
                            exp_emit(h, slice(512, 1024))
                    else:
                        for c in range(2):
                            for h in range(2):
                                score_mm(c, h)
                        for h in range(2):
                            exp_emit(h, slice(0, S))
                    # previous pair's PV chunk-1, spread over slots 0-3
                    if p > 0 and T <= 3:
                        if T == 0:
                            pv1_of[p - 1] = [bg_tile(f"pvb{h}") for h in range(2)]
                        emit_pv1_mm(p - 1, [2 * T, 2 * T + 1])
                        if T == 3:
                            evac_pv1(p - 1)
                    # last pair: own PV chunk-1 in slots 4-7 (bg banks are
                    # free -- no next pair's q/k to project)
                    if p == NPAIR - 1 and T >= 4:
                        if T == 4:
                            pv1_of[p] = [bg_tile(f"pvb{h}") for h in range(2)]
                        emit_pv1_mm(p, [2 * (T - 4), 2 * (T - 4) + 1])
                    for th in extras[p][T]:
                        th()
                    if T > 0:
                        for h in range(2):
                            nc.tensor.matmul(
                                pv0_of[p][h][:],
                                v_ext[T - 1][
                                    :, (2 * p + h) * VW:(2 * p + h + 1) * VW
                                ],
                                pts_of[p][T - 1][:, h * S:h * S + 512],
                                start=(T == 1),
                                stop=(T == NC_T - 1),
                            )

                def finish_c0(p):
                    # last chunk-0 PV matmul + chunk-0 evacuation
                    for h in range(2):
                        nc.tensor.matmul(
                            pv0_of[p][h][:],
                            v_ext[NC_T - 1][
                                :, (2 * p + h) * VW:(2 * p + h + 1) * VW
                            ],
                            pts_of[p][NC_T - 1][:, h * S:h * S + 512],
                            start=False,
                            stop=True,
                        )
                    for h in range(2):
                        nc.vector.tensor_copy(
                            sums_sb_of[p][0:1, h * S:h * S + 512],
                            pv0_of[p][h][HD:HD + 1, :],
                        )
                        nc.vector.tensor_copy(
                            attnT[p][h * 64:(h + 1) * 64, 0:512],
                            pv0_of[p][h][0:HD, :],
                        )
                    del pv0_of[p]
                    norm_half(p, 0)

                def norm_half(p, c):
                    # normalize attnT[p] columns [c*512, (c+1)*512):
                    # DVE fast reciprocal of the sums, then a partition-
                    # broadcast + one in-place multiply. Mid-stream pairs
                    # bounce the reciprocals through DRAM (Sync engine,
                    # hides under the PE); the last pair is on the tail
                    # critical path, so it broadcasts via an ones-matmul on
                    # the then-idle PE instead (saves the DMA round trips).
                    lo, hi = c * 512, (c + 1) * 512
                    rr = normp.tile([1, 1024], F32, tag="rr", name="rr", bufs=2)
                    for h in range(2):
                        # sums are O(100) positive fp32 -- no edge cases;
                        # ~51 ULP is far inside the fp16 noise floor
                        nc.vector.reciprocal_approx_fast(
                            out=rr[0:1, h * 512:(h + 1) * 512],
                            in_=sums_sb_of[p][0:1, h * S + lo:h * S + hi],
                        )
                    if p == NPAIR - 1:
                        bc = bg_tile("bc")
                        for h in range(2):
                            nc.tensor.matmul(
                                bc[h * 64:(h + 1) * 64, :],
                                ones_f32[0:1, :],
                                rr[0:1, h * 512:(h + 1) * 512],
                                start=True,
                                stop=True,
                            )
                        nc.vector.tensor_mul(
                            attnT[p][:, lo:hi], attnT[p][:, lo:hi], bc[:]
                        )
                        if c == 1:
                            del sums_sb_of[p]
                        return
                    rdd = rdram.tile([1, 1024], F32, tag="rdd", name="rdd")
                    nc.sync.dma_start(out=rdd[:], in_=rr[:])
                    rb = normp.tile([128, 512], F32, tag="rb", name="rb", bufs=2)
                    for h in range(2):
                        row = rdd[0:1, h * 512:(h + 1) * 512]
                        row_bc = bass.AP(
                            tensor=row.tensor,
                            offset=row.offset,
                            ap=[[0, 64]] + list(row.ap[1:]),
                        )
                        nc.sync.dma_start(
                            out=rb[h * 64:(h + 1) * 64, :], in_=row_bc
                        )
                    nc.vector.tensor_mul(
                        attnT[p][:, lo:hi], attnT[p][:, lo:hi], rb[:]
                    )
                    if c == 1:
                        del sums_sb_of[p]

                for p in range(NPAIR):
                    for T in range(NC_T):
                        slot(p, T)
                    finish_c0(p)
                # last pair: PV c1 already fully accumulated in slots 4-7
                evac_pv1(NPAIR - 1)

                # ---------------- output projection ----------------
                # PSUM comes from the score-tile tags (st0/st1), so the
                # first projection matmuls start the moment the last exp
                # frees them. Depth-2 pipeline: each tile's pair-5 matmul
                # (gated by the last normalization) is deferred past the
                # next tile's early matmuls so the PE never blocks on it.
                def proj_head(tt, h):
                    ps = stps.tile([128, DIM], F32, tag=f"st{h}",
                                   name=f"prj{tt}", bufs=1)
                    for o0, ow in [(0, 512), (512, 256)]:
                        for p in range(NPAIR - 1):
                            nc.tensor.matmul(
                                ps[:, o0:o0 + ow],
                                attnT[p][:, tt * 128:(tt + 1) * 128],
                                wprojT[:, p, o0:o0 + ow],
                                start=(p == 0),
                                stop=False,
                            )
                    return ps

                def proj_tail(tt, ps):
                    for o0, ow in [(0, 512), (512, 256)]:
                        nc.tensor.matmul(
                            ps[:, o0:o0 + ow],
                            attnT[NPAIR - 1][:, tt * 128:(tt + 1) * 128],
                            wprojT[:, NPAIR - 1, o0:o0 + ow],
                            start=False,
                            stop=True,
                        )
                    ob = outp.tile([128, DIM], FP16, tag="ob", name="ob")
                    nc.scalar.copy(out=ob[:], in_=ps[:])
                    nc.sync.dma_start(
                        out=out_ext[tt * 128:(tt + 1) * 128, :], in_=ob[:]
                    )

                pending = None
                for tt in range(NC_T):
                    ps = proj_head(tt, tt % 2)
                    if pending is not None:
                        proj_tail(*pending)
                    pending = (tt, ps)
                proj_tail(*pending)

    nc.finalize()
    return nc


_NC_CACHE = None


def kernel(**inputs) -> np.ndarray:
    global _NC_CACHE
    x = np.asarray(inputs["x"], dtype=np.float32)
    w_qkv = np.asarray(inputs["w_qkv"], dtype=np.float32)
    w_proj = np.asarray(inputs["w_proj"], dtype=np.float32)
    b_proj = np.asarray(inputs["b_proj"], dtype=np.float32)
    B, H, W, C = x.shape
    assert (B, H * W, C) == (8, S, DIM)

    # host-side sharding + layout prep: channel-major fp16 operands
    wqkvT = np.ascontiguousarray(w_qkv.T).astype(np.float16)       # [768, 2304]
    wprojT = np.ascontiguousarray(w_proj.T).astype(np.float16)     # [768, 768]
    xTs = [
        np.ascontiguousarray(x[b].reshape(S, DIM).T).astype(np.float16)
        for b in range(B)
    ]

    if _NC_CACHE is None:
        _NC_CACHE = build_bass()
    nc = _NC_CACHE

    in_maps = [
        {"xT": xTs[b], "w_qkvT": wqkvT, "w_projT": wprojT}
        for b in range(B)
    ]
    res = run_bass_kernel_spmd(nc, in_maps, list(range(B)))
    out = np.stack(
        [
            np.asarray(res.results[b]["out"]).astype(np.float32).reshape(H, W, C)
            for b in range(B)
        ]
    )
    return (out + b_proj.reshape(1, 1, 1, C)).astype(np.float32)


if __name__ == "__main__":
    rng = np.random.default_rng(0)
    ins = {
        "x": rng.standard_normal((8, 32, 32, DIM), dtype=np.float32),
        "w_qkv": rng.standard_normal((3 * DIM, DIM), dtype=np.float32)
        * DIM ** -0.5,
        "w_proj": rng.standard_normal((DIM, DIM), dtype=np.float32) * DIM ** -0.5,
        "b_proj": np.zeros(DIM, dtype=np.float32),
    }
    o = kernel(**ins)
    print(o.shape, o.dtype)


# revision 15
# speedup vs baseline: 1.0078x; 1.0008x over previous
"""Trainium2 Bass kernel for nn_Attention_10917806866815.

Multi-head attention forward (B=8, S=32x32=1024, C=768, 12 heads, hd=64),
data-parallel across 8 NeuronCores: core b computes batch element b.
No collectives needed.

Host side: inputs pre-transposed to channel-major fp16, so the device
kernel is pure matmul work: xT [768,1024], w_qkvT [768,2304],
w_projT [768,768].

Per-core pipeline, software-pipelined around the two binding engines
(TensorE matmul stream ~125us of N-cycles, ScalarE exp stream ~106us):
  - 8 consolidated 3D DMAs; a long zero-matmul warmup keeps the PE
    clock ramped through the DMA lead-in.
  - Per head pair p, per token-tile T (slot): 4 score matmuls (the two
    heads live at partition bases 0/64 so their K=64 matmuls run on
    disjoint PE row groups concurrently), 2 exps (ScalarE, straight out
    of PSUM; scores ~ N(0,1)*8 pre-scale so exp cannot overflow), then
    background PE work: PV chunk-0 at one-slot lag, the previous pair's
    PV chunk-1 spread over slots 0-3 (pt tiles are double-buffered so
    they survive into the next pair), and the next pair's q/k
    projections + v token-tiles as slot extras (slots 4-7).
  - PV uses 65-wide lhsT = [v_h | ones]: attn rows 0-63 plus softmax
    sums in row 64. Sums evacuate to a [2, S] tile on GpSimd (idle
    engine); attn rows on DVE.
  - Normalization per half: DVE reciprocal of [2,512] sums, bounce via
    DRAM for the partition-broadcast, one in-place multiply. No ScalarE
    table swap (keeps the exp table resident).
  - Output projection from the c-major attnT tiles, fp16 out DMA;
    bias is added host-side.

Precision: fp16 operands (10-bit mantissa) with fp32 PSUM accumulation.
"""

import numpy as np

import concourse.bass as bass
import concourse.mybir as mybir
import concourse.tile as tile
from concourse import bacc
from concourse.bass_utils import run_bass_kernel_spmd

DIM = 768
S = 1024
NH = 12
HD = 64
SCALE = HD ** -0.5

F32 = mybir.dt.float32
FP16 = mybir.dt.float16

NC_T = S // 128          # 8 token tiles
NC_C = DIM // 128        # 6 channel tiles
NPAIR = NH // 2          # 6 head pairs
VW = HD + 1              # 65: v columns per head incl. ones column

N_WARMUP = 32


def build_bass():
    nc = bacc.Bacc(None, target_bir_lowering=False)

    xT_ext = nc.declare_dram_parameter("xT", [DIM, S], FP16, isOutput=False)
    wqkvT_ext = nc.declare_dram_parameter(
        "w_qkvT", [DIM, 3 * DIM], FP16, isOutput=False
    )
    wprojT_ext = nc.declare_dram_parameter(
        "w_projT", [DIM, DIM], FP16, isOutput=False
    )
    out_ext = nc.declare_dram_parameter("out", [S, DIM], FP16, isOutput=True)

    with tile.TileContext(nc) as tc:
        from contextlib import ExitStack

        with ExitStack() as ctx:
            consts = ctx.enter_context(tc.tile_pool(name="consts", bufs=1))
            persist = ctx.enter_context(tc.tile_pool(name="persist", bufs=1))

            # c-major operands: [:, j, :] is channel-tile j.
            xT = persist.tile([128, NC_C, S], FP16, tag="xT", name="xT")
            wqkvT = persist.tile(
                [128, NC_C, 3 * DIM], FP16, tag="wqkvT", name="wqkvT"
            )
            wprojT = persist.tile([128, NC_C, DIM], FP16, tag="wprojT", name="wprojT")

            # warmup operands + fp32 ones row (PE-broadcast normalization of
            # the last pair); emitted before the DMAs so the warmup matmuls
            # start the moment the framework preamble ends
            wu = consts.tile([128, 512], FP16, tag="wu", name="wu")
            nc.vector.memset(wu[:], 0.0)
            ones_f32 = consts.tile([1, 64], F32, tag="ones_f32", name="ones_f32")
            nc.vector.memset(ones_f32[:], 1.0)
            # preload the exp activation table while DMAs stream in, so the
            # first real exp doesn't pay the ~1.3us ACT_TABLE_LOAD
            tl = consts.tile([1, 8], FP16, tag="tl", name="tl")
            nc.scalar.activation(
                out=tl[:], in_=wu[0:1, 0:8],
                func=mybir.ActivationFunctionType.Exp, scale=1.0,
            )

            # one 3D DMA per column range: dst [128, j, cols] <- src rows
            # (j*128 + p), ordered so pair-0's q/k data lands first
            def dma_cols(dst, src, src_w, lo, hi):
                src3 = src.rearrange("(j p) s -> p j s", p=128)
                nc.sync.dma_start(
                    out=dst[:, :, lo:hi],
                    in_=src3[:, :, lo:hi],
                )

            # lead-in data split per channel-tile so the first q/k
            # projection chain pipelines with the DMA stream
            dma_cols(wqkvT, wqkvT_ext, 3 * DIM, 0, 128)
            dma_cols(wqkvT, wqkvT_ext, 3 * DIM, DIM, DIM + 128)
            for j in range(NC_C):
                nc.sync.dma_start(
                    out=xT[:, j, 0:512],
                    in_=xT_ext[j * 128:(j + 1) * 128, 0:512],
                )
            dma_cols(xT, xT_ext, S, 512, 1024)
            dma_cols(wqkvT, wqkvT_ext, 3 * DIM, 2 * DIM, 3 * DIM)
            dma_cols(wqkvT, wqkvT_ext, 3 * DIM, 128, DIM)
            dma_cols(wqkvT, wqkvT_ext, 3 * DIM, DIM + 128, 2 * DIM)
            dma_cols(wprojT, wprojT_ext, DIM, 0, DIM)

            qkT = [
                persist.tile([128, S], FP16, tag=f"qkT{ot}", name=f"qkT{ot}")
                for ot in range(2 * NPAIR)
            ]
            v_ext = [
                persist.tile([128, NH * VW], FP16, tag=f"vext{tt}", name=f"vext{tt}")
                for tt in range(NC_T)
            ]
            attnT = [
                persist.tile([128, S], FP16, tag=f"attnT{p}", name=f"attnT{p}")
                for p in range(NPAIR)
            ]
            for tt in range(NC_T):
                nc.gpsimd.memset(v_ext[tt][:], 1.0)

            with (
                tc.tile_pool(name="stps", bufs=1, space="PSUM") as stps,
                tc.tile_pool(name="pvps", bufs=1, space="PSUM") as pvps,
                tc.tile_pool(name="bgps", bufs=1, space="PSUM") as bgps,
                tc.tile_pool(name="ptpool", bufs=1) as ptpool,
                tc.tile_pool(name="normp", bufs=2) as normp,
                tc.tile_pool(name="outp", bufs=3) as outp,
                tc.tile_pool(name="rdram", bufs=2, space="DRAM") as rdram,
            ):
                # background PSUM: two [128,512] banks time-shared by the
                # QKV-projection extras (slots 4-7) and the spread PV
                # chunk-1 accumulators (slots 0-3)
                bg_flip = [0]

                def bg_tile(name):
                    t = bgps.tile([128, 512], F32, tag=f"bg{bg_flip[0]}",
                                  name=name, bufs=1)
                    bg_flip[0] ^= 1
                    return t

                # ---- QKV building blocks (emitted as in-slot extras) ----
                def emit_qk_chunk(ot, c):
                    ps = bg_tile("qkvp")
                    for k in range(NC_C):
                        nc.tensor.matmul(
                            ps[:],
                            wqkvT[:, k, ot * 128:(ot + 1) * 128],
                            xT[:, k, c * 512:(c + 1) * 512],
                            start=(k == 0),
                            stop=(k == NC_C - 1),
                        )
                    nc.vector.tensor_copy(qkT[ot][:, c * 512:(c + 1) * 512], ps[:])

                def emit_v_chunk(tt, c):
                    o0, ow, h0, nh = [
                        (2 * DIM, 512, 0, 8), (2 * DIM + 512, 256, 8, 4)
                    ][c]
                    ps = bg_tile("vp")
                    for k in range(NC_C):
                        nc.tensor.matmul(
                            ps[:, :ow],
                            xT[:, k, tt * 128:(tt + 1) * 128],
                            wqkvT[:, k, o0:o0 + ow],
                            start=(k == 0),
                            stop=(k == NC_C - 1),
                        )
                    dst = (
                        v_ext[tt][:]
                        .rearrange("p (h e) -> p h e", e=VW)[:, h0:h0 + nh, 0:HD]
                    )
                    nc.vector.tensor_copy(
                        dst, ps[:, :ow].rearrange("p (h e) -> p h e", e=HD)
                    )

                # extras[p][T]: QKV work slotted into PE idle time under the
                # exp stream. Pair 0 carries its own v c0 tiles (slots 0-7)
                # plus pair 1's q/k; later pairs carry q/k for pair p+1 and
                # the v c1 tiles (slots 4-7 only -- the bg PSUM banks are
                # held by the spread PV chunk-1 during slots 0-3).
                extras = [[[] for _ in range(NC_T)] for _ in range(NPAIR)]

                def TH(f, *a):
                    return lambda: f(*a)

                for tt in range(NC_T):
                    extras[0][tt].append(TH(emit_v_chunk, tt, 0))
                extras[0][2].append(TH(emit_qk_chunk, NPAIR, 1))
                for p in range(NPAIR - 1):  # q/k for pair p+1
                    extras[p][4].append(TH(emit_qk_chunk, p + 1, 0))
                    extras[p][5].append(TH(emit_qk_chunk, p + 1, 1))
                    extras[p][6].append(TH(emit_qk_chunk, NPAIR + p + 1, 0))
                    extras[p][7].append(TH(emit_qk_chunk, NPAIR + p + 1, 1))
                # v chunk-1 tiles: pairs 1-3, slots 4-6 (needed by pair 4)
                for i in range(NC_T):
                    extras[1 + i // 3][4 + i % 3].append(TH(emit_v_chunk, i, 1))

                # ---- HAM warm-up: keep the PE busy through the DMA
                # lead-in so the first real matmuls run at full clock ----
                wups = bg_tile("wups")
                for i in range(N_WARMUP):
                    # N=128 keeps each warmup matmul short, so the first
                    # real projection matmul queues at fine granularity
                    # while the clock ramps through the DMA lead-in
                    nc.tensor.matmul(
                        wups[:, 0:128], wu[:, 0:128], wu[:, 0:128],
                        start=(i == 0), stop=(i == N_WARMUP - 1),
                    )

                # ---- pair-0 q/k projection, c0 only: slot (0,0)'s
                # T=0 scores touch kT cols 0-127 and both q halves; q c1 is
                # projected just-in-time inside the split slot, k c1 (first
                # needed at T=4) rides as a slot-2 extra ----
                emit_qk_chunk(0, 0)
                emit_qk_chunk(NPAIR, 0)

                # ---- attention: software-pipelined slot stream ----
                pts_of = {}     # p -> [pt]*8, pt cols = h*S + q
                pv0_of = {}     # p -> [pv0_h0, pv0_h1]
                pv1_of = {}     # p -> [pv1_h0, pv1_h1]
                sums_sb_of = {}  # p -> [2, S] sums tile (h, q)

                def emit_pv1_mm(pp, tprims):
                    # spread PV chunk-1 matmuls for pair pp
                    for h in range(2):
                        for T in tprims:
                            nc.tensor.matmul(
                                pv1_of[pp][h][0:VW, :],
                                v_ext[T][:, (2 * pp + h) * VW:(2 * pp + h + 1) * VW],
                                pts_of[pp][T][:, h * S + 512:h * S + 1024],
                                start=(T == 0),
                                stop=(T == NC_T - 1),
                            )

                def evac_pv1(pp):
                    for h in range(2):
                        nc.vector.tensor_copy(
                            sums_sb_of[pp][0:1, h * S + 512:h * S + 1024],
                            pv1_of[pp][h][HD:HD + 1, :],
                        )
                        nc.vector.tensor_copy(
                            attnT[pp][h * 64:(h + 1) * 64, 512:1024],
                            pv1_of[pp][h][0:HD, :],
                        )
                    del pv1_of[pp], pts_of[pp]
                    norm_half(pp, 1)

                def slot(p, T):
                    kT_t = qkT[NPAIR + p]
                    qT_t = qkT[p]
                    if T == 0:
                        sums_sb_of[p] = normp.tile(
                            [1, 2 * S], F32, tag="sums", name="sums", bufs=2
                        )
                        pv0_of[p] = [
                            pvps.tile([VW, 512], F32, tag=f"pva{h}",
                                      name=f"pva{h}", bufs=1)
                            for h in range(2)
                        ]
                        pts_of[p] = [None] * NC_T
                    # two per-h score tiles: h1's exp overlaps h0's
                    # next-slot scores (separate PSUM bank pairs)
                    st = [
                        stps.tile([128, S], F32, tag=f"st{h}", name=f"st{h}",
                                  bufs=1)
                        for h in range(2)
                    ]
                    pt = ptpool.tile(
                        [128, 2 * S], FP16, tag=f"pt{T}", name=f"pt{T}",
                        bufs=2
                    )
                    pts_of[p][T] = pt

                    def score_mm(c, h):
                        r0 = h * 64
                        # partition bases 0/64 -> disjoint PE row groups
                        # -> the two heads' score matmuls overlap
                        nc.tensor.matmul(
                            st[h][:, c * 512:(c + 1) * 512],
                            kT_t[r0:r0 + 64, T * 128:(T + 1) * 128],
                            qT_t[r0:r0 + 64, c * 512:(c + 1) * 512],
                            start=True,
                            stop=True,
                        )

                    def exp_emit(h, cols):
                        nc.scalar.activation(
                            out=pt[:, h * S + cols.start:h * S + cols.stop],
                            in_=st[h][:, cols],
                            func=mybir.ActivationFunctionType.Exp,
                            scale=float(SCALE),
                        )

                    if p == 0 and T == 0:
                        # pipeline-fill slot: per-half exps so ScalarE
                        # starts before the q c1 projection exists
                        for h in range(2):
                            score_mm(0, h)
                        for h in range(2):
                            exp_emit(h, slice(0, 512))
                        emit_qk_chunk(0, 1)
                        for h in range(2):
                            score_mm(1, h)
                        for h in range(2):
                            exp_emit(h, slice(512, 1024))
                    else:
                        for c in range(2):
                            for h in range(2):
                                score_mm(c, h)
                        for h in range(2):
                            exp_emit(h, slice(0, S))
                    # previous pair's PV chunk-1, spread over slots 0-3
                    if p > 0 and T <= 3:
                        if T == 0:
                            pv1_of[p - 1] = [bg_tile(f"pvb{h}") for h in range(2)]
                        emit_pv1_mm(p - 1, [2 * T, 2 * T + 1])
                        if T == 3:
                            evac_pv1(p - 1)
                    # last pair: own PV chunk-1 in slots 4-7 (bg banks are
                    # free -- no next pair's q/k to project)
                    if p == NPAIR - 1 and T >= 4:
                        if T == 4:
                            pv1_of[p] = [bg_tile(f"pvb{h}") for h in range(2)]
                        emit_pv1_mm(p, [2 * (T - 4), 2 * (T - 4) + 1])
                    for th in extras[p][T]:
                        th()
                    if T > 0:
                        for h in range(2):
                            nc.tensor.matmul(
                                pv0_of[p][h][:],
                                v_ext[T - 1][
                                    :, (2 * p + h) * VW:(2 * p + h + 1) * VW
                                ],
                                pts_of[p][T - 1][:, h * S:h * S + 512],
                                start=(T == 1),
                                stop=(T == NC_T - 1),
                            )

                def finish_c0(p):
                    # last chunk-0 PV matmul + chunk-0 evacuation
                    for h in range(2):
                        nc.tensor.matmul(
                            pv0_of[p][h][:],
                            v_ext[NC_T - 1][
                                :, (2 * p + h) * VW:(2 * p + h + 1) * VW
                            ],
                            pts_of[p][NC_T - 1][:, h * S:h * S + 512],
                            start=False,
                            stop=True,
                        )
                    for h in range(2):
                        nc.vector.tensor_copy(
                            sums_sb_of[p][0:1, h * S:h * S + 512],
                            pv0_of[p][h][HD:HD + 1, :],
                        )
                        nc.vector.tensor_copy(
                            attnT[p][h * 64:(h + 1) * 64, 0:512],
                            pv0_of[p][h][0:HD, :],
                        )
                    del pv0_of[p]
                    norm_half(p, 0)

                def norm_half(p, c):
                    # normalize attnT[p] columns [c*512, (c+1)*512):
                    # DVE fast reciprocal of the sums, then a partition-
                    # broadcast + one in-place multiply. Mid-stream pairs
                    # bounce the reciprocals through DRAM (Sync engine,
                    # hides under the PE); the last pair is on the tail
                    # critical path, so it broadcasts via an ones-matmul on
                    # the then-idle PE instead (saves the DMA round trips).
                    lo, hi = c * 512, (c + 1) * 512
                    rr = normp.tile([1, 1024], F32, tag="rr", name="rr", bufs=2)
                    for h in range(2):
                        # sums are O(100) positive fp32 -- no edge cases;
                        # ~51 ULP is far inside the fp16 noise floor
                        nc.vector.reciprocal_approx_fast(
                            out=rr[0:1, h * 512:(h + 1) * 512],
                            in_=sums_sb_of[p][0:1, h * S + lo:h * S + hi],
                        )
                    if p == NPAIR - 1:
                        bc = bg_tile("bc")
                        for h in range(2):
                            nc.tensor.matmul(
                                bc[h * 64:(h + 1) * 64, :],
                                ones_f32[0:1, :],
                                rr[0:1, h * 512:(h + 1) * 512],
                                start=True,
                                stop=True,
                            )
                        nc.vector.tensor_mul(
                            attnT[p][:, lo:hi], attnT[p][:, lo:hi], bc[:]
                        )
                        if c == 1:
                            del sums_sb_of[p]
                        return
                    rdd = rdram.tile([1, 1024], F32, tag="rdd", name="rdd")
                    nc.sync.dma_start(out=rdd[:], in_=rr[:])
                    rb = normp.tile([128, 512], F32, tag="rb", name="rb", bufs=2)
                    for h in range(2):
                        row = rdd[0:1, h * 512:(h + 1) * 512]
                        row_bc = bass.AP(
                            tensor=row.tensor,
                            offset=row.offset,
                            ap=[[0, 64]] + list(row.ap[1:]),
                        )
                        nc.sync.dma_start(
                            out=rb[h * 64:(h + 1) * 64, :], in_=row_bc
                        )
                    nc.vector.tensor_mul(
                        attnT[p][:, lo:hi], attnT[p][:, lo:hi], rb[:]
                    )
                    if c == 1:
                        del sums_sb_of[p]

                for p in range(NPAIR):
                    for T in range(NC_T):
                        slot(p, T)
                    finish_c0(p)
                # last pair: PV c1 already fully accumulated in slots 4-7
                evac_pv1(NPAIR - 1)

                # ---------------- output projection ----------------
                # PSUM comes from the score-tile tags (st0/st1), so the
                # first projection matmuls start the moment the last exp
                # frees them. Depth-2 pipeline: each tile's pair-5 matmul
                # (gated by the last normalization) is deferred past the
                # next tile's early matmuls so the PE never blocks on it.
                def proj_head(tt, h):
                    ps = stps.tile([128, DIM], F32, tag=f"st{h}",
                                   name=f"prj{tt}", bufs=1)
                    for o0, ow in [(0, 512), (512, 256)]:
                        for p in range(NPAIR - 1):
                            nc.tensor.matmul(
                                ps[:, o0:o0 + ow],
                                attnT[p][:, tt * 128:(tt + 1) * 128],
                                wprojT[:, p, o0:o0 + ow],
                                start=(p == 0),
                                stop=False,
                            )
                    return ps

                def proj_tail(tt, ps):
                    for o0, ow in [(0, 512), (512, 256)]:
                        nc.tensor.matmul(
                            ps[:, o0:o0 + ow],
                            attnT[NPAIR - 1][:, tt * 128:(tt + 1) * 128],
                            wprojT[:, NPAIR - 1, o0:o0 + ow],
                            start=False,
                            stop=True,
                        )
                    ob = outp.tile([128, DIM], FP16, tag="ob", name="ob")
                    nc.scalar.copy(out=ob[:], in_=ps[:])
                    nc.sync.dma_start(
                        out=out_ext[tt * 128:(tt + 1) * 128, :], in_=ob[:]
                    )

                pending = None
                for tt in range(NC_T):
                    ps = proj_head(tt, tt % 2)
                    if pending is not None:
                        proj_tail(*pending)
                    pending = (tt, ps)
                proj_tail(*pending)

    nc.finalize()
    return nc


_NC_CACHE = None


def kernel(**inputs) -> np.ndarray:
    global _NC_CACHE
    x = np.asarray(inputs["x"], dtype=np.float32)
    w_qkv = np.asarray(inputs["w_qkv"], dtype=np.float32)
    w_proj = np.asarray(inputs["w_proj"], dtype=np.float32)
    b_proj = np.asarray(inputs["b_proj"], dtype=np.float32)
    B, H, W, C = x.shape
    assert (B, H * W, C) == (8, S, DIM)

    # host-side sharding + layout prep: channel-major fp16 operands
    wqkvT = np.ascontiguousarray(w_qkv.T).astype(np.float16)       # [768, 2304]
    wprojT = np.ascontiguousarray(w_proj.T).astype(np.float16)     # [768, 768]
    xTs = [
        np.ascontiguousarray(x[b].reshape(S, DIM).T).astype(np.float16)
        for b in range(B)
    ]

    if _NC_CACHE is None:
        _NC_CACHE = build_bass()
    nc = _NC_CACHE

    in_maps = [
        {"xT": xTs[b], "w_qkvT": wqkvT, "w_projT": wprojT}
        for b in range(B)
    ]
    res = run_bass_kernel_spmd(nc, in_maps, list(range(B)))
    out = np.stack(
        [
            np.asarray(res.results[b]["out"]).astype(np.float32).reshape(H, W, C)
            for b in range(B)
        ]
    )
    return (out + b_proj.reshape(1, 1, 1, C)).astype(np.float32)


if __name__ == "__main__":
    rng = np.random.default_rng(0)
    ins = {
        "x": rng.standard_normal((8, 32, 32, DIM), dtype=np.float32),
        "w_qkv": rng.standard_normal((3 * DIM, DIM), dtype=np.float32)
        * DIM ** -0.5,
        "w_proj": rng.standard_normal((DIM, DIM), dtype=np.float32) * DIM ** -0.5,
        "b_proj": np.zeros(DIM, dtype=np.float32),
    }
    o = kernel(**ins)
    print(o.shape, o.dtype)
